# revision 1
# baseline (speedup 1.0000x reference)
"""Trainium2 Bass kernel for the DeepFace-style CNN (nn_DeepFace_10574209482846).

Sharding: pure data parallel — batch 2048 split as 256 images per core
across 8 cores; all weights replicated (host-preprocessed into matmul-
friendly block-diagonal / stacked layouts, cast to bf16).

Per-core layout: the 256 images form 4 "groups" of 64. Dense convs run
with channels on partitions and (b, y, x) on the free dim, 4 groups (or
2-group pairs) stacked on partitions via block-diagonal weights so the
128-wide PE array is filled.  Locally-connected layers use a
batch-contiguous (y, x, b) layout with a +1-column-shifted replica on
partitions 64..127 so two kernel taps contract per matmul (K=128).
"""

import numpy as np
import concourse.bass as bass
import concourse.bacc as bacc
import concourse.tile as tile
import concourse.mybir as mybir
from concourse import bass_utils

bf16 = mybir.dt.bfloat16
f32 = mybir.dt.float32
BF = mybir.dt.np(bf16)  # ml_dtypes.bfloat16

N_CORES = 8
B_FULL = 2048
B_CORE = 256          # images per core
SB = 8                # images per sub-batch (2 per group)
NSB = B_CORE // SB    # 32
BSB = SB // 4         # 2 images per group per sub-batch

TAPS3 = [(di, dj) for di in range(3) for dj in range(3)]

_CACHE = {}


def _build_module(nsb=NSB, phase2=True):
    nc = bacc.Bacc("TRN2", target_bir_lowering=False, debug=False,
                   enable_asserts=True, num_devices=N_CORES)

    # ---- DRAM I/O ----
    x_d = nc.dram_tensor("x", [B_CORE, 5, 3600], bf16, kind="ExternalInput").ap()
    w1bd_d = nc.dram_tensor("w1bd", [20, 9 * 128], bf16, kind="ExternalInput").ap()
    b1t_d = nc.dram_tensor("b1t", [128, 1], f32, kind="ExternalInput").ap()
    w2abd_d = nc.dram_tensor("w2abd", [128, 9 * 128], bf16, kind="ExternalInput").ap()
    b2at_d = nc.dram_tensor("b2at", [128, 1], f32, kind="ExternalInput").ap()
    w2bbd_d = nc.dram_tensor("w2bbd", [128, 9 * 128], bf16, kind="ExternalInput").ap()
    b2bt_d = nc.dram_tensor("b2bt", [128, 1], f32, kind="ExternalInput").ap()
    lw3p_d = nc.dram_tensor("lw3p", [81, 128, 640], bf16, kind="ExternalInput").ap()
    lw3s_d = nc.dram_tensor("lw3s", [81, 64, 320], bf16, kind="ExternalInput").ap()
    lb3_d = nc.dram_tensor("lb3t", [64, 81], f32, kind="ExternalInput").ap()
    lw4p_d = nc.dram_tensor("lw4p", [25, 128, 640], bf16, kind="ExternalInput").ap()
    lw4s_d = nc.dram_tensor("lw4s", [25, 64, 320], bf16, kind="ExternalInput").ap()
    lb4_d = nc.dram_tensor("lb4t", [64, 25], f32, kind="ExternalInput").ap()
    lw5p_d = nc.dram_tensor("lw5p", [9, 128, 192], bf16, kind="ExternalInput").ap()
    lw5s_d = nc.dram_tensor("lw5s", [9, 64, 192], bf16, kind="ExternalInput").ap()
    lb5_d = nc.dram_tensor("lb5t", [64, 9], f32, kind="ExternalInput").ap()
    hwch_d = nc.dram_tensor("hwch", [64, 18], bf16, kind="ExternalInput").ap()
    logits_d = nc.dram_tensor("logits", [2, B_CORE], f32, kind="ExternalOutput").ap()

    Tanh = mybir.ActivationFunctionType.Tanh

    with tile.TileContext(nc) as tc:
        with (
            tc.tile_pool(name="wp", bufs=1) as wp,
            tc.tile_pool(name="lwp", bufs=3) as lwp,
            tc.tile_pool(name="xp", bufs=1) as xp,
            tc.tile_pool(name="h1p", bufs=1) as h1p,
            tc.tile_pool(name="h2ap", bufs=1) as h2ap,
            tc.tile_pool(name="bigp", bufs=1) as bigp,
            tc.tile_pool(name="cps", bufs=4, space="PSUM") as cps,
            tc.tile_pool(name="lps", bufs=3, space="PSUM") as lps,
            tc.tile_pool(name="hps", bufs=1, space="PSUM") as hps,
        ):
            # ---- persistent weights ----
            w1bd = wp.tile([20, 9 * 128], bf16)
            nc.sync.dma_start(w1bd[:], w1bd_d[:])
            b1t = wp.tile([128, 1], f32)
            nc.sync.dma_start(b1t[:], b1t_d[:])
            w2abd = wp.tile([128, 9 * 128], bf16)
            nc.sync.dma_start(w2abd[:], w2abd_d[:])
            b2at = wp.tile([128, 1], f32)
            nc.sync.dma_start(b2at[:], b2at_d[:])
            w2bbd = wp.tile([128, 9 * 128], bf16)
            nc.sync.dma_start(w2bbd[:], w2bbd_d[:])
            b2bt = wp.tile([128, 1], f32)
            nc.sync.dma_start(b2bt[:], b2bt_d[:])
            lb3t = wp.tile([64, 81], f32)
            nc.sync.dma_start(lb3t[:], lb3_d[:])
            lb4t = wp.tile([64, 25], f32)
            nc.sync.dma_start(lb4t[:], lb4_d[:])
            lb5t = wp.tile([64, 9], f32)
            nc.sync.dma_start(lb5t[:], lb5_d[:])
            hwch = wp.tile([64, 18], bf16)
            nc.sync.dma_start(hwch[:], hwch_d[:])

            # ---- persistent activations (batch-contiguous, (y, x, b)) ----
            h2brep = bigp.tile([128, 169 * 256], bf16)   # rows 0-63 h2b, 64-127 +1col
            h3rep = bigp.tile([128, 81 * 256], bf16)
            h4rep = bigp.tile([128, 25 * 256], bf16)
            h5t = bigp.tile([64, 9 * 256], bf16)

            h2bv = h2brep[:].rearrange("c (y x b) -> c b y x", y=13, x=13, b=256)

            # ================= phase 1: conv1 -> conv2a -> conv2b =============
            for sb in range(nsb):
                # load x sub-batch: 4 groups stacked at partition rows 5g
                x_t = xp.tile([20, BSB * 3600], bf16, tag="x")
                for g in range(4):
                    b0 = 64 * g + BSB * sb
                    src = x_d[b0:b0 + BSB, :, :].rearrange("b c m -> c b m")
                    nc.sync.dma_start(
                        x_t[5 * g:5 * g + 5, :].rearrange("c (b m) -> c b m", b=BSB),
                        src)
                xv = x_t[:].rearrange("c (b h w) -> c b h w", b=BSB, h=60, w=60)

                # ---- conv1: K=20 block-diag over 4 groups, M=128 = 4x32co ----
                h1_t = h1p.tile([128, BSB * 841], bf16, tag="h1")
                h1v = h1_t[:].rearrange("c (b h w) -> c b h w", b=BSB, h=29, w=29)
                for (y0, ny) in [(0, 8), (8, 8), (16, 8), (24, 5)]:
                    ps = cps.tile([128, BSB * 8 * 29], f32, tag="cps")
                    psw = ps[:, :BSB * ny * 29]
                    for t, (di, dj) in enumerate(TAPS3):
                        rhs = xv[:, :, 2 * y0 + di: 2 * y0 + di + 2 * ny - 1: 2,
                                 dj: dj + 57: 2]
                        nc.tensor.matmul(psw, w1bd[:, 128 * t:128 * (t + 1)], rhs,
                                         start=(t == 0), stop=(t == 8))
                    nc.scalar.activation(h1v[:, :, y0:y0 + ny, :], psw, Tanh,
                                         bias=b1t[:])

                # ---- conv2a: 2 pairs x (K=64 block-diag), M=128 = 2x64co ----
                h2a_t = {}
                for r in range(2):  # pair r: groups (2r, 2r+1), lhsT rows 64r..
                    h2a_t[r] = h2ap.tile([128, BSB * 729], bf16, tag=f"h2a{r}", name=f"h2a{r}")
                    h2av = h2a_t[r][:].rearrange("c (b h w) -> c b h w",
                                                 b=BSB, h=27, w=27)
                    for (y0, ny) in [(0, 9), (9, 9), (18, 9)]:
                        ps = cps.tile([128, BSB * 9 * 27], f32, tag="cps")
                        psw = ps[:, :BSB * ny * 27]
                        for t, (di, dj) in enumerate(TAPS3):
                            rhs = h1v[64 * r:64 * (r + 1), :,
                                      y0 + di: y0 + di + ny, dj: dj + 27]
                            nc.tensor.matmul(
                                psw,
                                w2abd[64 * r:64 * (r + 1), 128 * t:128 * (t + 1)],
                                rhs, start=(t == 0), stop=(t == 8))
                        nc.scalar.activation(h2av[:, :, y0:y0 + ny, :], psw, Tanh,
                                             bias=b2at[:])

                # ---- conv2b: per pair, K=128 block-diag, stride 2 ----
                for r in range(2):
                    h2av = h2a_t[r][:].rearrange("c (b h w) -> c b h w",
                                                 b=BSB, h=27, w=27)
                    ps = cps.tile([128, BSB * 169], f32, tag="cps")
                    for t, (di, dj) in enumerate(TAPS3):
                        rhs = h2av[:, :, di: di + 25: 2, dj: dj + 25: 2]
                        nc.tensor.matmul(ps[:], w2bbd[:, 128 * t:128 * (t + 1)],
                                         rhs, start=(t == 0), stop=(t == 8))
                    # evacuate: psum rows (g-even 0:64 / g-odd 64:128) into
                    # h2brep[0:64, (y, x, b)] at the right global-b columns
                    psv = ps[:].rearrange("c (b y x) -> c b y x", b=BSB, y=13, x=13)
                    for g2 in range(2):
                        gb = 64 * (2 * r + g2) + BSB * sb
                        nc.scalar.activation(
                            h2bv[0:64, gb:gb + BSB, :, :],
                            psv[64 * g2:64 * (g2 + 1), :, :, :], Tanh,
                            bias=b2bt[64 * g2:64 * (g2 + 1)])

            # replica rows 64-127 = +1 x-column shift (= +256 elements)
            nc.vector.tensor_copy(h2brep[64:128, 0:169 * 256 - 256],
                                  h2brep[0:64, 256:169 * 256])

            if not phase2:
                lg = wp.tile([2, 256], f32, name="lg")
                nc.vector.tensor_copy(lg[:], h2brep[0:2, 0:512].bitcast(f32))
                nc.sync.dma_start(logits_d[:], lg[:])
            if phase2:
                # ================= phase 2: locally-connected stack ===============
                # ---- lconv3: 13x13 -> 9x9, 5x5 taps ----
                for p in range(81):
                    i, j = p // 9, p % 9
                    lwt = lwp.tile([128, 640], bf16, tag="lw3p")
                    nc.sync.dma_start(lwt[:], lw3p_d[p])
                    lws = lwp.tile([64, 320], bf16, tag="lw3s")
                    nc.sync.dma_start(lws[:], lw3s_d[p])
                    ps = lps.tile([64, 256], f32, tag="lps")
                    for q in range(10):        # (u, v0/v0+1) pairs, K=128
                        u, v0 = q // 2, 2 * (q % 2)
                        col = ((i + u) * 13 + (j + v0)) * 256
                        nc.tensor.matmul(ps[:], lwt[:, 64 * q:64 * (q + 1)],
                                         h2brep[:, col:col + 256],
                                         start=(q == 0), stop=False)
                    for s in range(5):         # (u, v=4) singles, K=64
                        col = ((i + s) * 13 + (j + 4)) * 256
                        nc.tensor.matmul(ps[:], lws[:, 64 * s:64 * (s + 1)],
                                         h2brep[0:64, col:col + 256],
                                         start=False, stop=(s == 4))
                    nc.scalar.activation(h3rep[0:64, 256 * p:256 * (p + 1)], ps[:],
                                         Tanh, bias=lb3t[:, p:p + 1])
                nc.vector.tensor_copy(h3rep[64:128, 0:81 * 256 - 256],
                                      h3rep[0:64, 256:81 * 256])

                # ---- lconv4: 9x9 -> 5x5, 5x5 taps ----
                for p in range(25):
                    i, j = p // 5, p % 5
                    lwt = lwp.tile([128, 640], bf16, tag="lw4p")
                    nc.sync.dma_start(lwt[:], lw4p_d[p])
                    lws = lwp.tile([64, 320], bf16, tag="lw4s")
                    nc.sync.dma_start(lws[:], lw4s_d[p])
                    ps = lps.tile([64, 256], f32, tag="lps")
                    for q in range(10):
                        u, v0 = q // 2, 2 * (q % 2)
                        col = ((i + u) * 9 + (j + v0)) * 256
                        nc.tensor.matmul(ps[:], lwt[:, 64 * q:64 * (q + 1)],
                                         h3rep[:, col:col + 256],
                                         start=(q == 0), stop=False)
                    for s in range(5):
                        col = ((i + s) * 9 + (j + 4)) * 256
                        nc.tensor.matmul(ps[:], lws[:, 64 * s:64 * (s + 1)],
                                         h3rep[0:64, col:col + 256],
                                         start=False, stop=(s == 4))
                    nc.scalar.activation(h4rep[0:64, 256 * p:256 * (p + 1)], ps[:],
                                         Tanh, bias=lb4t[:, p:p + 1])
                nc.vector.tensor_copy(h4rep[64:128, 0:25 * 256 - 256],
                                      h4rep[0:64, 256:25 * 256])

                # ---- lconv5: 5x5 -> 3x3, 3x3 taps ----
                for p in range(9):
                    i, j = p // 3, p % 3
                    lwt = lwp.tile([128, 192], bf16, tag="lw5p")
                    nc.sync.dma_start(lwt[:], lw5p_d[p])
                    lws = lwp.tile([64, 192], bf16, tag="lw5s")
                    nc.sync.dma_start(lws[:], lw5s_d[p])
                    ps = lps.tile([64, 256], f32, tag="lps")
                    for q in range(3):         # (u, v=0/1) pairs
                        col = ((i + q) * 5 + (j + 0)) * 256
                        nc.tensor.matmul(ps[:], lwt[:, 64 * q:64 * (q + 1)],
                                         h4rep[:, col:col + 256],
                                         start=(q == 0), stop=False)
                    for s in range(3):         # (u, v=2) singles
                        col = ((i + s) * 5 + (j + 2)) * 256
                        nc.tensor.matmul(ps[:], lws[:, 64 * s:64 * (s + 1)],
                                         h4rep[0:64, col:col + 256],
                                         start=False, stop=(s == 2))
                    nc.scalar.activation(h5t[:, 256 * p:256 * (p + 1)], ps[:],
                                         Tanh, bias=lb5t[:, p:p + 1])

                # ---- head: logits[o, b] = sum_f hw[o, f] feat[f, b] (h5 part) ----
                psh = hps.tile([2, 256], f32)
                for yx in range(9):
                    nc.tensor.matmul(psh[:], hwch[:, 2 * yx:2 * yx + 2],
                                     h5t[:, 256 * yx:256 * (yx + 1)],
                                     start=(yx == 0), stop=(yx == 8))
                lg = wp.tile([2, 256], f32)
                nc.vector.tensor_copy(lg[:], psh[:])
                nc.sync.dma_start(logits_d[:], lg[:])

    nc.compile()
    return nc


def _prep_weights(w1, b1, w2a, b2a, w2b, b2b, lw3, lb3, lw4, lb4, lw5, lb5, hw):
    """Host-side reshape of weights into the on-chip matmul layouts."""
    out = {}
    w1bd = np.zeros((20, 9, 128), np.float32)
    for t, (di, dj) in enumerate(TAPS3):
        blk = w1[:, :, di, dj].T          # [5ci, 32co]
        for g in range(4):
            w1bd[5 * g:5 * g + 5, t, 32 * g:32 * g + 32] = blk
    out["w1bd"] = w1bd.reshape(20, 9 * 128).astype(BF)
    out["b1t"] = np.tile(b1, 4)[:, None].astype(np.float32)

    w2abd = np.zeros((128, 9, 128), np.float32)
    for t, (di, dj) in enumerate(TAPS3):
        blk = w2a[:, :, di, dj].T         # [32ci, 64co]
        for r in range(2):                # strip copy for pair-B at rows 64+
            for g2 in range(2):
                w2abd[64 * r + 32 * g2:64 * r + 32 * (g2 + 1), t,
                      64 * g2:64 * (g2 + 1)] = blk
    out["w2abd"] = w2abd.reshape(128, 9 * 128).astype(BF)
    out["b2at"] = np.tile(b2a, 2)[:, None].astype(np.float32)

    w2bbd = np.zeros((128, 9, 128), np.float32)
    for t, (di, dj) in enumerate(TAPS3):
        blk = w2b[:, :, di, dj].T         # [64ci, 64co]
        for g2 in range(2):
            w2bbd[64 * g2:64 * (g2 + 1), t, 64 * g2:64 * (g2 + 1)] = blk
    out["w2bbd"] = w2bbd.reshape(128, 9 * 128).astype(BF)
    out["b2bt"] = np.tile(b2b, 2)[:, None].astype(np.float32)

    def lc_pack(lw, Ho, Wo, kh, kw):
        npos = Ho * Wo
        npair = kh * (kw // 2)
        nsing = kh
        lp = np.zeros((npos, 128, npair * 64), np.float32)
        ls = np.zeros((npos, 64, nsing * 64), np.float32)
        for p in range(npos):
            i, j = p // Wo, p % Wo
            for q in range(npair):
                u, v0 = q // (kw // 2), 2 * (q % (kw // 2))
                lp[p, 0:64, 64 * q:64 * (q + 1)] = lw[i, j, :, :, u, v0].T
                lp[p, 64:128, 64 * q:64 * (q + 1)] = lw[i, j, :, :, u, v0 + 1].T
            for s in range(nsing):
                ls[p, 0:64, 64 * s:64 * (s + 1)] = lw[i, j, :, :, s, kw - 1].T
        return lp.astype(BF), ls.astype(BF)

    out["lw3p"], out["lw3s"] = lc_pack(lw3, 9, 9, 5, 5)
    out["lb3t"] = np.ascontiguousarray(
        lb3.transpose(2, 0, 1).reshape(64, 81)).astype(np.float32)
    out["lw4p"], out["lw4s"] = lc_pack(lw4, 5, 5, 5, 5)
    out["lb4t"] = np.ascontiguousarray(
        lb4.transpose(2, 0, 1).reshape(64, 25)).astype(np.float32)
    out["lw5p"], out["lw5s"] = lc_pack(lw5, 3, 3, 3, 3)
    out["lb5t"] = np.ascontiguousarray(
        lb5.transpose(2, 0, 1).reshape(64, 9)).astype(np.float32)

    # head: feature f = co*9 + yx; chunk yx -> [64co, 2]
    out["hwch"] = np.ascontiguousarray(
        hw[:, :576].reshape(2, 64, 9).transpose(1, 2, 0).reshape(64, 18)
    ).astype(BF)
    return out


def kernel(x, info, w1, b1, w2a, b2a, w2b, b2b, lw3, lb3, lw4, lb4, lw5, lb5,
           hw, hb, _trace=False):
    x = np.asarray(x, np.float32)
    if "nc" not in _CACHE:
        _CACHE["nc"] = _build_module()
    nc = _CACHE["nc"]

    wts = _prep_weights(np.asarray(w1, np.float32), np.asarray(b1, np.float32),
                        np.asarray(w2a, np.float32), np.asarray(b2a, np.float32),
                        np.asarray(w2b, np.float32), np.asarray(b2b, np.float32),
                        np.asarray(lw3, np.float32), np.asarray(lb3, np.float32),
                        np.asarray(lw4, np.float32), np.asarray(lb4, np.float32),
                        np.asarray(lw5, np.float32), np.asarray(lb5, np.float32),
                        np.asarray(hw, np.float32))

    xb = np.ascontiguousarray(x.reshape(B_FULL, 5, 3600)).astype(BF)
    in_maps = []
    for c in range(N_CORES):
        m = dict(wts)
        m["x"] = xb[c * B_CORE:(c + 1) * B_CORE]
        in_maps.append(m)

    res = bass_utils.run_bass_kernel_spmd(
        nc, in_maps, core_ids=list(range(N_CORES)), trace=_trace)
    if _trace:
        _CACHE["last_results"] = res

    logits = np.concatenate(
        [res.results[c]["logits"].T for c in range(N_CORES)], axis=0)  # [2048, 2]

    # host-side tail: info contribution + bias + softmax (fp32)
    info = np.asarray(info, np.float32)
    hw = np.asarray(hw, np.float32)
    hb = np.asarray(hb, np.float32)
    logits = logits + info @ hw[:, 576:].T + hb[None, :]
    m = logits.max(axis=1, keepdims=True)
    e = np.exp(logits - m)
    return (e / e.sum(axis=1, keepdims=True)).astype(np.float32)



# revision 3
# speedup vs baseline: 1.0376x; 1.0376x over previous
"""Trainium2 Bass kernel for the DeepFace-style CNN (nn_DeepFace_10574209482846).

Sharding: pure data parallel — batch 2048 split as 256 images per core
across 8 cores; all weights replicated (host-preprocessed into matmul-
friendly layouts, cast to bf16).

v2: phase 1 (dense convs) restructured for sustained PE clock (HAM 8/8):
 - x host-preprocessed into even/odd column parity planes so the
   stride-2 conv1 reads contiguous runs; conv1 contracts tap-pairs
   (dj=0,1) via the two parity planes stacked on partitions (K=40).
 - activations stored (y, x, b)-inner so matmul rhs streams are long
   contiguous runs (few AP wraps -> no PE dead time).
 - conv2a contracts tap-pairs via a +1-column-shifted replica of h1 on
   partitions 64..127 (K=128), 9 taps -> 3 pairs + 3 singles.
 - h2a stored with parity-split columns so stride-2 conv2b reads runs.
Phase 2 (locally-connected stack) unchanged: batch-contiguous (y, x, b)
layout with +1-column-shifted replica, 2 taps per matmul (K=128).
"""

import numpy as np
import concourse.bass as bass
import concourse.bacc as bacc
import concourse.tile as tile
import concourse.mybir as mybir
from concourse import bass_utils

bf16 = mybir.dt.bfloat16
f32 = mybir.dt.float32
BF = mybir.dt.np(bf16)  # ml_dtypes.bfloat16

N_CORES = 8
B_FULL = 2048
B_CORE = 256          # images per core
SB = 8                # images per sub-batch (2 per group)
NSB = B_CORE // SB    # 32
BSB = SB // 4         # 2 images per group per sub-batch

TAPS3 = [(di, dj) for di in range(3) for dj in range(3)]

_CACHE = {}


def _build_module(nsb=NSB):
    nc = bacc.Bacc("TRN2", target_bir_lowering=False, debug=False,
                   enable_asserts=True, num_devices=N_CORES)

    # ---- DRAM I/O ----
    x_d = nc.dram_tensor("x", [B_CORE, 2, 5, 1800], bf16, kind="ExternalInput").ap()
    w1p_d = nc.dram_tensor("w1p", [40, 3 * 128], bf16, kind="ExternalInput").ap()
    w1s_d = nc.dram_tensor("w1s", [20, 3 * 128], bf16, kind="ExternalInput").ap()
    b1t_d = nc.dram_tensor("b1t", [128, 1], f32, kind="ExternalInput").ap()
    w2ap_d = nc.dram_tensor("w2ap", [128, 3 * 128], bf16, kind="ExternalInput").ap()
    w2as_d = nc.dram_tensor("w2as", [64, 3 * 128], bf16, kind="ExternalInput").ap()
    b2at_d = nc.dram_tensor("b2at", [128, 1], f32, kind="ExternalInput").ap()
    w2bbd_d = nc.dram_tensor("w2bbd", [128, 9 * 128], bf16, kind="ExternalInput").ap()
    b2bt_d = nc.dram_tensor("b2bt", [128, 1], f32, kind="ExternalInput").ap()
    lw3p_d = nc.dram_tensor("lw3p", [81, 128, 640], bf16, kind="ExternalInput").ap()
    lw3s_d = nc.dram_tensor("lw3s", [81, 64, 320], bf16, kind="ExternalInput").ap()
    lb3_d = nc.dram_tensor("lb3t", [64, 81], f32, kind="ExternalInput").ap()
    lw4p_d = nc.dram_tensor("lw4p", [25, 128, 640], bf16, kind="ExternalInput").ap()
    lw4s_d = nc.dram_tensor("lw4s", [25, 64, 320], bf16, kind="ExternalInput").ap()
    lb4_d = nc.dram_tensor("lb4t", [64, 25], f32, kind="ExternalInput").ap()
    lw5p_d = nc.dram_tensor("lw5p", [9, 128, 192], bf16, kind="ExternalInput").ap()
    lw5s_d = nc.dram_tensor("lw5s", [9, 64, 192], bf16, kind="ExternalInput").ap()
    lb5_d = nc.dram_tensor("lb5t", [64, 9], f32, kind="ExternalInput").ap()
    hwch_d = nc.dram_tensor("hwch", [64, 18], bf16, kind="ExternalInput").ap()
    logits_d = nc.dram_tensor("logits", [2, B_CORE], f32, kind="ExternalOutput").ap()

    Tanh = mybir.ActivationFunctionType.Tanh

    with tile.TileContext(nc) as tc:
        with (
            tc.tile_pool(name="wp", bufs=1) as wp,
            tc.tile_pool(name="lwp", bufs=3) as lwp,
            tc.tile_pool(name="xp", bufs=2) as xp,
            tc.tile_pool(name="h1p", bufs=1) as h1p,
            tc.tile_pool(name="h2ap", bufs=1) as h2ap,
            tc.tile_pool(name="bigp", bufs=1) as bigp,
            tc.tile_pool(name="cps", bufs=4, space="PSUM") as cps,
            tc.tile_pool(name="lps", bufs=3, space="PSUM") as lps,
            tc.tile_pool(name="hps", bufs=1, space="PSUM") as hps,
        ):
            # ---- persistent weights ----
            w1p = wp.tile([40, 3 * 128], bf16)
            nc.sync.dma_start(w1p[:], w1p_d[:])
            w1s = wp.tile([20, 3 * 128], bf16)
            nc.sync.dma_start(w1s[:], w1s_d[:])
            b1t = wp.tile([128, 1], f32)
            nc.sync.dma_start(b1t[:], b1t_d[:])
            w2ap = wp.tile([128, 3 * 128], bf16)
            nc.sync.dma_start(w2ap[:], w2ap_d[:])
            w2as = wp.tile([64, 3 * 128], bf16)
            nc.sync.dma_start(w2as[:], w2as_d[:])
            b2at = wp.tile([128, 1], f32)
            nc.sync.dma_start(b2at[:], b2at_d[:])
            w2bbd = wp.tile([128, 9 * 128], bf16)
            nc.sync.dma_start(w2bbd[:], w2bbd_d[:])
            b2bt = wp.tile([128, 1], f32)
            nc.sync.dma_start(b2bt[:], b2bt_d[:])
            lb3t = wp.tile([64, 81], f32)
            nc.sync.dma_start(lb3t[:], lb3_d[:])
            lb4t = wp.tile([64, 25], f32)
            nc.sync.dma_start(lb4t[:], lb4_d[:])
            lb5t = wp.tile([64, 9], f32)
            nc.sync.dma_start(lb5t[:], lb5_d[:])
            hwch = wp.tile([64, 18], bf16)
            nc.sync.dma_start(hwch[:], hwch_d[:])

            # ---- persistent activations (batch-contiguous, (y, x, b)) ----
            h2brep = bigp.tile([128, 169 * 256], bf16)   # rows 0-63 h2b, 64-127 +1col
            h3rep = bigp.tile([128, 81 * 256], bf16)
            h4rep = bigp.tile([128, 25 * 256], bf16)
            h5t = bigp.tile([64, 9 * 256], bf16)

            h2bv = h2brep[:].rearrange("c (y x b) -> c y x b", y=13, x=13, b=256)

            # ============ phase 1: conv1 -> conv2a -> conv2b ============
            for sb in range(nsb):
                # x sub-batch: partitions = par*20 + g*5 + ci
                x_t = xp.tile([40, BSB * 1800], bf16, tag="x")
                for g in range(4):
                    b0 = 64 * g + BSB * sb
                    for par in range(2):
                        src = x_d[b0:b0 + BSB, par, :, :].rearrange("b c m -> c b m")
                        nc.sync.dma_start(
                            x_t[20 * par + 5 * g:20 * par + 5 * g + 5, :]
                            .rearrange("c (b m) -> c b m", b=BSB), src)
                xv = x_t[:].rearrange("c (b h w) -> c b h w", b=BSB, h=60, w=30)

                # ---- conv1: tap-pairs via parity planes; K=40/20, M=128 ----
                # h1main: (y, x, b) layout, rows = 4g x 32co
                h1m = h1p.tile([128, 1682], bf16, tag="h1m")
                h1mv = h1m[:].rearrange("c (y x b) -> c b y x", y=29, x=29, b=BSB)
                for (y0, ny) in [(0, 8), (8, 8), (16, 8), (24, 5)]:
                    ps = cps.tile([128, BSB * 9 * 27], f32, tag="cps")
                    psw = ps[:, :BSB * ny * 29]
                    for di in range(3):
                        rows = slice(2 * y0 + di, 2 * y0 + di + 2 * ny - 1, 2)
                        nc.tensor.matmul(psw, w1p[:, 128 * di:128 * (di + 1)],
                                         xv[0:40, :, rows, 0:29],
                                         start=(di == 0), stop=False)
                        nc.tensor.matmul(psw, w1s[:, 128 * di:128 * (di + 1)],
                                         xv[0:20, :, rows, 1:30],
                                         start=False, stop=(di == 2))
                    nc.scalar.activation(h1mv[:, :, y0:y0 + ny, :], psw, Tanh,
                                         bias=b1t[:])

                # ---- build pair tiles with +1-x shifted replica (DVE) ----
                h1pr = {}
                for r in range(2):
                    h1pr[r] = h1p.tile([128, 1682], bf16, tag=f"h1pr{r}",
                                       name=f"h1pr{r}")
                    base = h1m[64 * r:64 * (r + 1), :]
                    nc.vector.tensor_copy(h1pr[r][0:64, :], base)
                    srcv = base.rearrange("c (y x b) -> c y x b", y=29, x=29, b=BSB)
                    dstv = h1pr[r][64:128, :].rearrange(
                        "c (y x b) -> c y x b", y=29, x=29, b=BSB)
                    nc.vector.tensor_copy(dstv[:, :, 0:28, :], srcv[:, :, 1:29, :])

                # ---- conv2a: 3 tap-pair MMs (K=128) + 3 singles (K=64) ----
                h2a_t = {}
                for r in range(2):
                    hp = h1pr[r][:].rearrange("c (y x b) -> c y x b",
                                              y=29, x=29, b=BSB)
                    # h2a layout: (y, par, xi, b): col = y*56 + par*28 + xi*2 + b
                    h2a_t[r] = h2ap.tile([128, 27 * 56], bf16, tag=f"h2a{r}",
                                         name=f"h2a{r}")
                    h2av = h2a_t[r][:].rearrange("c (y p xi b) -> c y p xi b",
                                                 y=27, p=2, xi=14, b=BSB)
                    for (y0, ny) in [(0, 9), (9, 9), (18, 9)]:
                        ps = cps.tile([128, BSB * 9 * 27], f32, tag="cps")
                        for di in range(3):
                            nc.tensor.matmul(
                                ps[:], w2ap[:, 128 * di:128 * (di + 1)],
                                hp[0:128, y0 + di:y0 + di + ny, 0:27, :],
                                start=(di == 0), stop=False)
                            nc.tensor.matmul(
                                ps[:], w2as[:, 128 * di:128 * (di + 1)],
                                hp[0:64, y0 + di:y0 + di + ny, 2:29, :],
                                start=False, stop=(di == 2))
                        psv = ps[:].rearrange("c (y x b) -> c y x b",
                                              y=ny, x=27, b=BSB)
                        nc.scalar.activation(
                            h2av[:, y0:y0 + ny, 0, 0:14, :],
                            psv[:, :, 0:27:2, :], Tanh, bias=b2at[:])
                        nc.scalar.activation(
                            h2av[:, y0:y0 + ny, 1, 0:13, :],
                            psv[:, :, 1:27:2, :], Tanh, bias=b2at[:])

                # ---- conv2b: stride 2, K=128 block-diag, 9 taps ----
                for r in range(2):
                    h2av = h2a_t[r][:].rearrange("c (y p xi b) -> c y p xi b",
                                                 y=27, p=2, xi=14, b=BSB)
                    ps = cps.tile([128, BSB * 9 * 27], f32, tag="cps")
                    psw = ps[:, :BSB * 169]
                    for t, (di, dj) in enumerate(TAPS3):
                        par, xi0 = dj % 2, dj // 2
                        rhs = h2av[:, di:di + 25:2, par, xi0:xi0 + 13, :]
                        nc.tensor.matmul(psw, w2bbd[:, 128 * t:128 * (t + 1)],
                                         rhs, start=(t == 0), stop=(t == 8))
                    psv = psw.rearrange("c (y x b) -> c y x b", y=13, x=13, b=BSB)
                    for g2 in range(2):
                        gb = 64 * (2 * r + g2) + BSB * sb
                        nc.scalar.activation(
                            h2bv[0:64, :, :, gb:gb + BSB],
                            psv[64 * g2:64 * (g2 + 1), :, :, :], Tanh,
                            bias=b2bt[64 * g2:64 * (g2 + 1)])

            # replica rows 64-127 = +1 x-column shift (= +256 elements)
            nc.vector.tensor_copy(h2brep[64:128, 0:169 * 256 - 256],
                                  h2brep[0:64, 256:169 * 256])

            # ============ phase 2: locally-connected stack ============
            # ---- lconv3: 13x13 -> 9x9, 5x5 taps ----
            for p in range(81):
                i, j = p // 9, p % 9
                lwt = lwp.tile([128, 640], bf16, tag="lw3p")
                nc.sync.dma_start(lwt[:], lw3p_d[p])
                lws = lwp.tile([64, 320], bf16, tag="lw3s")
                nc.sync.dma_start(lws[:], lw3s_d[p])
                ps = lps.tile([64, 256], f32, tag="lps")
                for q in range(10):        # (u, v0/v0+1) pairs, K=128
                    u, v0 = q // 2, 2 * (q % 2)
                    col = ((i + u) * 13 + (j + v0)) * 256
                    nc.tensor.matmul(ps[:], lwt[:, 64 * q:64 * (q + 1)],
                                     h2brep[:, col:col + 256],
                                     start=(q == 0), stop=False)
                for s in range(5):         # (u, v=4) singles, K=64
                    col = ((i + s) * 13 + (j + 4)) * 256
                    nc.tensor.matmul(ps[:], lws[:, 64 * s:64 * (s + 1)],
                                     h2brep[0:64, col:col + 256],
                                     start=False, stop=(s == 4))
                nc.scalar.activation(h3rep[0:64, 256 * p:256 * (p + 1)], ps[:],
                                     Tanh, bias=lb3t[:, p:p + 1])
            nc.vector.tensor_copy(h3rep[64:128, 0:81 * 256 - 256],
                                  h3rep[0:64, 256:81 * 256])

            # ---- lconv4: 9x9 -> 5x5, 5x5 taps ----
            for p in range(25):
                i, j = p // 5, p % 5
                lwt = lwp.tile([128, 640], bf16, tag="lw4p")
                nc.sync.dma_start(lwt[:], lw4p_d[p])
                lws = lwp.tile([64, 320], bf16, tag="lw4s")
                nc.sync.dma_start(lws[:], lw4s_d[p])
                ps = lps.tile([64, 256], f32, tag="lps")
                for q in range(10):
                    u, v0 = q // 2, 2 * (q % 2)
                    col = ((i + u) * 9 + (j + v0)) * 256
                    nc.tensor.matmul(ps[:], lwt[:, 64 * q:64 * (q + 1)],
                                     h3rep[:, col:col + 256],
                                     start=(q == 0), stop=False)
                for s in range(5):
                    col = ((i + s) * 9 + (j + 4)) * 256
                    nc.tensor.matmul(ps[:], lws[:, 64 * s:64 * (s + 1)],
                                     h3rep[0:64, col:col + 256],
                                     start=False, stop=(s == 4))
                nc.scalar.activation(h4rep[0:64, 256 * p:256 * (p + 1)], ps[:],
                                     Tanh, bias=lb4t[:, p:p + 1])
            nc.vector.tensor_copy(h4rep[64:128, 0:25 * 256 - 256],
                                  h4rep[0:64, 256:25 * 256])

            # ---- lconv5: 5x5 -> 3x3, 3x3 taps ----
            for p in range(9):
                i, j = p // 3, p % 3
                lwt = lwp.tile([128, 192], bf16, tag="lw5p")
                nc.sync.dma_start(lwt[:], lw5p_d[p])
                lws = lwp.tile([64, 192], bf16, tag="lw5s")
                nc.sync.dma_start(lws[:], lw5s_d[p])
                ps = lps.tile([64, 256], f32, tag="lps")
                for q in range(3):         # (u, v=0/1) pairs
                    col = ((i + q) * 5 + (j + 0)) * 256
                    nc.tensor.matmul(ps[:], lwt[:, 64 * q:64 * (q + 1)],
                                     h4rep[:, col:col + 256],
                                     start=(q == 0), stop=False)
                for s in range(3):         # (u, v=2) singles
                    col = ((i + s) * 5 + (j + 2)) * 256
                    nc.tensor.matmul(ps[:], lws[:, 64 * s:64 * (s + 1)],
                                     h4rep[0:64, col:col + 256],
                                     start=False, stop=(s == 2))
                nc.scalar.activation(h5t[:, 256 * p:256 * (p + 1)], ps[:],
                                     Tanh, bias=lb5t[:, p:p + 1])

            # ---- head: logits[o, b] = sum_f hw[o, f] feat[f, b] (h5 part) ----
            psh = hps.tile([2, 256], f32)
            for yx in range(9):
                nc.tensor.matmul(psh[:], hwch[:, 2 * yx:2 * yx + 2],
                                 h5t[:, 256 * yx:256 * (yx + 1)],
                                 start=(yx == 0), stop=(yx == 8))
            lg = wp.tile([2, 256], f32)
            nc.vector.tensor_copy(lg[:], psh[:])
            nc.sync.dma_start(logits_d[:], lg[:])

    nc.compile()
    return nc


def _prep_weights(w1, b1, w2a, b2a, w2b, b2b, lw3, lb3, lw4, lb4, lw5, lb5, hw):
    """Host-side reshape of weights into the on-chip matmul layouts."""
    out = {}
    # conv1 tap-pair weights: K rows = par*20 + g*5 + ci, cols g*32 + co
    w1p = np.zeros((3, 40, 128), np.float32)
    w1s = np.zeros((3, 20, 128), np.float32)
    for di in range(3):
        for g in range(4):
            for par in range(2):
                w1p[di, 20 * par + 5 * g:20 * par + 5 * g + 5,
                    32 * g:32 * (g + 1)] = w1[:, :, di, par].T
            w1s[di, 5 * g:5 * g + 5, 32 * g:32 * (g + 1)] = w1[:, :, di, 2].T
    out["w1p"] = np.ascontiguousarray(
        w1p.transpose(1, 0, 2).reshape(40, 3 * 128)).astype(BF)
    out["w1s"] = np.ascontiguousarray(
        w1s.transpose(1, 0, 2).reshape(20, 3 * 128)).astype(BF)
    out["b1t"] = np.tile(b1, 4)[:, None].astype(np.float32)

    # conv2a tap-pair weights: rows rep*64 + gl*32 + ci, cols gl*64 + co
    w2ap = np.zeros((3, 128, 128), np.float32)
    w2as = np.zeros((3, 64, 128), np.float32)
    for di in range(3):
        for gl in range(2):
            for rep in range(2):
                w2ap[di, 64 * rep + 32 * gl:64 * rep + 32 * (gl + 1),
                     64 * gl:64 * (gl + 1)] = w2a[:, :, di, rep].T
            w2as[di, 32 * gl:32 * (gl + 1),
                 64 * gl:64 * (gl + 1)] = w2a[:, :, di, 2].T
    out["w2ap"] = np.ascontiguousarray(
        w2ap.transpose(1, 0, 2).reshape(128, 3 * 128)).astype(BF)
    out["w2as"] = np.ascontiguousarray(
        w2as.transpose(1, 0, 2).reshape(64, 3 * 128)).astype(BF)
    out["b2at"] = np.tile(b2a, 2)[:, None].astype(np.float32)

    w2bbd = np.zeros((128, 9, 128), np.float32)
    for t, (di, dj) in enumerate(TAPS3):
        blk = w2b[:, :, di, dj].T         # [64ci, 64co]
        for g2 in range(2):
            w2bbd[64 * g2:64 * (g2 + 1), t, 64 * g2:64 * (g2 + 1)] = blk
    out["w2bbd"] = w2bbd.reshape(128, 9 * 128).astype(BF)
    out["b2bt"] = np.tile(b2b, 2)[:, None].astype(np.float32)

    def lc_pack(lw, Ho, Wo, kh, kw):
        npos = Ho * Wo
        npair = kh * (kw // 2)
        nsing = kh
        lp = np.zeros((npos, 128, npair * 64), np.float32)
        ls = np.zeros((npos, 64, nsing * 64), np.float32)
        for p in range(npos):
            i, j = p // Wo, p % Wo
            for q in range(npair):
                u, v0 = q // (kw // 2), 2 * (q % (kw // 2))
                lp[p, 0:64, 64 * q:64 * (q + 1)] = lw[i, j, :, :, u, v0].T
                lp[p, 64:128, 64 * q:64 * (q + 1)] = lw[i, j, :, :, u, v0 + 1].T
            for s in range(nsing):
                ls[p, 0:64, 64 * s:64 * (s + 1)] = lw[i, j, :, :, s, kw - 1].T
        return lp.astype(BF), ls.astype(BF)

    out["lw3p"], out["lw3s"] = lc_pack(lw3, 9, 9, 5, 5)
    out["lb3t"] = np.ascontiguousarray(
        lb3.transpose(2, 0, 1).reshape(64, 81)).astype(np.float32)
    out["lw4p"], out["lw4s"] = lc_pack(lw4, 5, 5, 5, 5)
    out["lb4t"] = np.ascontiguousarray(
        lb4.transpose(2, 0, 1).reshape(64, 25)).astype(np.float32)
    out["lw5p"], out["lw5s"] = lc_pack(lw5, 3, 3, 3, 3)
    out["lb5t"] = np.ascontiguousarray(
        lb5.transpose(2, 0, 1).reshape(64, 9)).astype(np.float32)

    # head: feature f = co*9 + yx; chunk yx -> [64co, 2]
    out["hwch"] = np.ascontiguousarray(
        hw[:, :576].reshape(2, 64, 9).transpose(1, 2, 0).reshape(64, 18)
    ).astype(BF)
    return out


def kernel(x, info, w1, b1, w2a, b2a, w2b, b2b, lw3, lb3, lw4, lb4, lw5, lb5,
           hw, hb, _trace=False):
    x = np.asarray(x, np.float32)
    if "nc" not in _CACHE:
        _CACHE["nc"] = _build_module()
    nc = _CACHE["nc"]

    wts = _prep_weights(np.asarray(w1, np.float32), np.asarray(b1, np.float32),
                        np.asarray(w2a, np.float32), np.asarray(b2a, np.float32),
                        np.asarray(w2b, np.float32), np.asarray(b2b, np.float32),
                        np.asarray(lw3, np.float32), np.asarray(lb3, np.float32),
                        np.asarray(lw4, np.float32), np.asarray(lb4, np.float32),
                        np.asarray(lw5, np.float32), np.asarray(lb5, np.float32),
                        np.asarray(hw, np.float32))

    # parity-split x: [B, 2par, 5ci, 60h, 30wi] -> [B, 2, 5, 1800]
    xb = np.ascontiguousarray(
        x.reshape(B_FULL, 5, 60, 30, 2).transpose(0, 4, 1, 2, 3)
        .reshape(B_FULL, 2, 5, 1800)).astype(BF)
    in_maps = []
    for c in range(N_CORES):
        m = dict(wts)
        m["x"] = xb[c * B_CORE:(c + 1) * B_CORE]
        in_maps.append(m)

    res = bass_utils.run_bass_kernel_spmd(
        nc, in_maps, core_ids=list(range(N_CORES)), trace=_trace)
    if _trace:
        _CACHE["last_results"] = res

    logits = np.concatenate(
        [res.results[c]["logits"].T for c in range(N_CORES)], axis=0)  # [2048, 2]

    # host-side tail: info contribution + bias + softmax (fp32)
    info = np.asarray(info, np.float32)
    hw = np.asarray(hw, np.float32)
    hb = np.asarray(hb, np.float32)
    logits = logits + info @ hw[:, 576:].T + hb[None, :]
    m = logits.max(axis=1, keepdims=True)
    e = np.exp(logits - m)
    return (e / e.sum(axis=1, keepdims=True)).astype(np.float32)


# revision 4
# speedup vs baseline: 2.0786x; 2.0032x over previous
"""Trainium2 Bass kernel for the DeepFace-style CNN (nn_DeepFace_10574209482846).

Sharding: pure data parallel — batch 2048 split as 256 images per core
across 8 cores; all weights replicated (host-preprocessed into matmul-
friendly layouts, cast to bf16).

v3 design notes (all driven by NTFF trace analysis):
 - Every phase-1 matmul is a uniform K=128 x M=128 shape (weights
   zero-padded).  Mixed LDWEIGHTS shapes serialize the weight loads
   (+~110ns/MM); uniform shapes let the PE background weight buffer
   hide them completely.  Streaming cost only depends on N, so K
   padding is free.
 - conv1 uses a 4-quadrant (h/w parity) packing of x prepared on the
   host: the 9 stride-2 taps collapse into 4 matmuls whose rhs are
   contiguous runs.  One DMA per sub-batch loads the pre-arranged
   [128, 1800] tile.
 - conv2a contracts tap-pairs via a +1-column-shifted replica of h1 on
   partitions 64..127 (K=128): 3 pair MMs + 3 (zero-padded) single MMs.
 - Activations are stored (y, x, b)-inner; h2a parity-split in x so the
   stride-2 conv2b streams contiguous 26-element runs.
 - The sub-batch loop is software-pipelined [conv1(i) | conv2b(i-1) |
   conv2a(i)] so the PE never stalls on the ACT-evac/DVE-replica chain
   (those bubbles re-throttled the PE clock (HAM) every iteration).
Phase 2 (locally-connected stack) unchanged except zero-padded single
taps (uniform K=128): batch-contiguous (y, x, b) layout with
+1-column-shifted replica, 2 taps per matmul.
"""

import numpy as np
import concourse.bass as bass
import concourse.bacc as bacc
import concourse.tile as tile
import concourse.mybir as mybir
from concourse import bass_utils

bf16 = mybir.dt.bfloat16
f32 = mybir.dt.float32
BF = mybir.dt.np(bf16)  # ml_dtypes.bfloat16

N_CORES = 8
B_FULL = 2048
B_CORE = 256          # images per core
SB = 8                # images per sub-batch (2 per group)
NSB = B_CORE // SB    # 32
BSB = SB // 4         # 2 images per group per sub-batch

TAPS3 = [(di, dj) for di in range(3) for dj in range(3)]
# conv1 quadrant planes: (hpar, wpar); P4 duplicates P0 (h0w0)
Q_PLANES = [(0, 0), (0, 1), (1, 1), (1, 0), (0, 0)]

_CACHE = {}


def _build_module(nsb=NSB):
    nc = bacc.Bacc("TRN2", target_bir_lowering=False, debug=False,
                   enable_asserts=True, num_devices=N_CORES)

    # ---- DRAM I/O ----
    xq_d = nc.dram_tensor("xq", [NSB, 128, 1800], bf16, kind="ExternalInput").ap()
    w1q_d = nc.dram_tensor("w1q", [128, 4 * 128], bf16, kind="ExternalInput").ap()
    b1t_d = nc.dram_tensor("b1t", [128, 1], f32, kind="ExternalInput").ap()
    w2ap_d = nc.dram_tensor("w2ap", [128, 3 * 128], bf16, kind="ExternalInput").ap()
    w2as_d = nc.dram_tensor("w2as", [128, 3 * 128], bf16, kind="ExternalInput").ap()
    b2at_d = nc.dram_tensor("b2at", [128, 1], f32, kind="ExternalInput").ap()
    w2bbd_d = nc.dram_tensor("w2bbd", [128, 9 * 128], bf16, kind="ExternalInput").ap()
    b2bt_d = nc.dram_tensor("b2bt", [128, 1], f32, kind="ExternalInput").ap()
    lw3p_d = nc.dram_tensor("lw3p", [81, 128, 640], bf16, kind="ExternalInput").ap()
    lw3s_d = nc.dram_tensor("lw3s", [81, 128, 320], bf16, kind="ExternalInput").ap()
    lb3_d = nc.dram_tensor("lb3t", [64, 81], f32, kind="ExternalInput").ap()
    lw4p_d = nc.dram_tensor("lw4p", [25, 128, 640], bf16, kind="ExternalInput").ap()
    lw4s_d = nc.dram_tensor("lw4s", [25, 128, 320], bf16, kind="ExternalInput").ap()
    lb4_d = nc.dram_tensor("lb4t", [64, 25], f32, kind="ExternalInput").ap()
    lw5p_d = nc.dram_tensor("lw5p", [9, 128, 192], bf16, kind="ExternalInput").ap()
    lw5s_d = nc.dram_tensor("lw5s", [9, 128, 192], bf16, kind="ExternalInput").ap()
    lb5_d = nc.dram_tensor("lb5t", [64, 9], f32, kind="ExternalInput").ap()
    hwch_d = nc.dram_tensor("hwch", [64, 18], bf16, kind="ExternalInput").ap()
    logits_d = nc.dram_tensor("logits", [2, B_CORE], f32, kind="ExternalOutput").ap()

    Tanh = mybir.ActivationFunctionType.Tanh

    with tile.TileContext(nc) as tc:
        with (
            tc.tile_pool(name="wp", bufs=1) as wp,
            tc.tile_pool(name="lwp", bufs=3) as lwp,
            tc.tile_pool(name="xp", bufs=2) as xp,
            tc.tile_pool(name="h1p", bufs=1) as h1p,
            tc.tile_pool(name="h2ap", bufs=2) as h2ap,
            tc.tile_pool(name="bigp", bufs=1) as bigp,
            tc.tile_pool(name="cps", bufs=4, space="PSUM") as cps,
            tc.tile_pool(name="lps", bufs=3, space="PSUM") as lps,
            tc.tile_pool(name="hps", bufs=1, space="PSUM") as hps,
        ):
            # ---- persistent weights ----
            w1q = wp.tile([128, 4 * 128], bf16)
            nc.sync.dma_start(w1q[:], w1q_d[:])
            b1t = wp.tile([128, 1], f32)
            nc.sync.dma_start(b1t[:], b1t_d[:])
            w2ap = wp.tile([128, 3 * 128], bf16)
            nc.sync.dma_start(w2ap[:], w2ap_d[:])
            w2as = wp.tile([128, 3 * 128], bf16)
            nc.sync.dma_start(w2as[:], w2as_d[:])
            b2at = wp.tile([128, 1], f32)
            nc.sync.dma_start(b2at[:], b2at_d[:])
            w2bbd = wp.tile([128, 9 * 128], bf16)
            nc.sync.dma_start(w2bbd[:], w2bbd_d[:])
            b2bt = wp.tile([128, 1], f32)
            nc.sync.dma_start(b2bt[:], b2bt_d[:])
            lb3t = wp.tile([64, 81], f32)
            nc.sync.dma_start(lb3t[:], lb3_d[:])
            lb4t = wp.tile([64, 25], f32)
            nc.sync.dma_start(lb4t[:], lb4_d[:])
            lb5t = wp.tile([64, 9], f32)
            nc.sync.dma_start(lb5t[:], lb5_d[:])
            hwch = wp.tile([64, 18], bf16)
            nc.sync.dma_start(hwch[:], hwch_d[:])

            # pair tiles for conv2a input: rows 0-63 base, 64-127 +1x shift.
            # Persistent; replica col 28 (x=28 shifted) is never a real tap
            # input (only read through zero-padded weights) — memset once so
            # it stays finite.
            h1pr = [wp.tile([128, 1682], bf16, name=f"h1pr{r}") for r in range(2)]
            for r in range(2):
                nc.vector.memset(h1pr[r][:], 0)

            # ---- persistent activations (batch-contiguous, (y, x, b)) ----
            h2brep = bigp.tile([128, 169 * 256], bf16)   # rows 0-63 h2b, 64-127 +1col
            h3rep = bigp.tile([128, 81 * 256], bf16)
            h4rep = bigp.tile([128, 25 * 256], bf16)
            h5t = bigp.tile([64, 9 * 256], bf16)

            h2bv = h2brep[:].rearrange("c (y x b) -> c y x b", y=13, x=13, b=256)

            # ============ phase 1 (pipelined): conv1(i) | conv2b(i-1) | conv2a(i)
            h2a_live = {}

            def conv1_stage(sb):
                x_t = xp.tile([128, BSB * 900], bf16, tag="x")
                nc.sync.dma_start(x_t[:], xq_d[sb])
                xv = x_t[:].rearrange("c (b h w) -> c b h w", b=BSB, h=30, w=30)

                h1m = h1p.tile([128, 1682], bf16, tag="h1m")
                h1mv = h1m[:].rearrange("c (y x b) -> c b y x", y=29, x=29, b=BSB)
                for (y0, ny) in [(0, 8), (8, 8), (16, 8), (24, 5)]:
                    ps = cps.tile([128, BSB * 9 * 27], f32, tag="cps")
                    psw = ps[:, :BSB * ny * 29]
                    # 4 uniform K=128 MMs (quadrant-packed taps)
                    nc.tensor.matmul(psw, w1q[:, 0:128],
                                     xv[:, :, y0:y0 + ny, 0:29],
                                     start=True, stop=False)
                    nc.tensor.matmul(psw, w1q[:, 128:256],
                                     xv[:, :, y0 + 1:y0 + 1 + ny, 0:29],
                                     start=False, stop=False)
                    nc.tensor.matmul(psw, w1q[:, 256:384],
                                     xv[:, :, y0:y0 + ny, 1:30],
                                     start=False, stop=False)
                    nc.tensor.matmul(psw, w1q[:, 384:512],
                                     xv[:, :, y0 + 1:y0 + 1 + ny, 1:30],
                                     start=False, stop=True)
                    nc.scalar.activation(h1mv[:, :, y0:y0 + ny, :], psw, Tanh,
                                         bias=b1t[:])
                    # replicas for this yblock (DVE, overlaps next MMs)
                    for r in range(2):
                        base = h1m[64 * r:64 * (r + 1), :]
                        srcv = base.rearrange("c (y x b) -> c y x b",
                                              y=29, x=29, b=BSB)
                        dstb = h1pr[r][0:64, :].rearrange(
                            "c (y x b) -> c y x b", y=29, x=29, b=BSB)
                        dsts = h1pr[r][64:128, :].rearrange(
                            "c (y x b) -> c y x b", y=29, x=29, b=BSB)
                        nc.vector.tensor_copy(dstb[:, y0:y0 + ny, :, :],
                                              srcv[:, y0:y0 + ny, :, :])
                        nc.vector.tensor_copy(dsts[:, y0:y0 + ny, 0:28, :],
                                              srcv[:, y0:y0 + ny, 1:29, :])

            def conv2a_stage(sb):
                h2a_t = {}
                for r in range(2):
                    hp = h1pr[r][:].rearrange("c (y x b) -> c y x b",
                                              y=29, x=29, b=BSB)
                    # h2a layout: (y, par, xi, b): col = y*56 + par*28 + xi*2 + b
                    h2a_t[r] = h2ap.tile([128, 27 * 56], bf16, tag=f"h2a{r}",
                                         name=f"h2a{r}_{sb}")
                    h2av = h2a_t[r][:].rearrange("c (y p xi b) -> c y p xi b",
                                                 y=27, p=2, xi=14, b=BSB)
                    for (y0, ny) in [(0, 9), (9, 9), (18, 9)]:
                        ps = cps.tile([128, BSB * 9 * 27], f32, tag="cps")
                        for di in range(3):
                            nc.tensor.matmul(
                                ps[:], w2ap[:, 128 * di:128 * (di + 1)],
                                hp[:, y0 + di:y0 + di + ny, 0:27, :],
                                start=(di == 0), stop=False)
                            nc.tensor.matmul(
                                ps[:], w2as[:, 128 * di:128 * (di + 1)],
                                hp[:, y0 + di:y0 + di + ny, 2:29, :],
                                start=False, stop=(di == 2))
                        psv = ps[:].rearrange("c (y x b) -> c y x b",
                                              y=ny, x=27, b=BSB)
                        nc.scalar.activation(
                            h2av[:, y0:y0 + ny, 0, 0:14, :],
                            psv[:, :, 0:27:2, :], Tanh, bias=b2at[:])
                        nc.scalar.activation(
                            h2av[:, y0:y0 + ny, 1, 0:13, :],
                            psv[:, :, 1:27:2, :], Tanh, bias=b2at[:])
                h2a_live[sb] = h2a_t

            def conv2b_stage(sb):
                h2a_t = h2a_live.pop(sb)
                for r in range(2):
                    h2av = h2a_t[r][:].rearrange("c (y p xi b) -> c y p xi b",
                                                 y=27, p=2, xi=14, b=BSB)
                    ps = cps.tile([128, BSB * 9 * 27], f32, tag="cps")
                    psw = ps[:, :BSB * 169]
                    for t, (di, dj) in enumerate(TAPS3):
                        par, xi0 = dj % 2, dj // 2
                        rhs = h2av[:, di:di + 25:2, par, xi0:xi0 + 13, :]
                        nc.tensor.matmul(psw, w2bbd[:, 128 * t:128 * (t + 1)],
                                         rhs, start=(t == 0), stop=(t == 8))
                    psv = psw.rearrange("c (y x b) -> c y x b", y=13, x=13, b=BSB)
                    for g2 in range(2):
                        gb = 64 * (2 * r + g2) + BSB * sb
                        nc.scalar.activation(
                            h2bv[0:64, :, :, gb:gb + BSB],
                            psv[64 * g2:64 * (g2 + 1), :, :, :], Tanh,
                            bias=b2bt[64 * g2:64 * (g2 + 1)])

            for it in range(nsb + 1):
                if it < nsb:
                    conv1_stage(it)
                if it >= 1:
                    conv2b_stage(it - 1)
                if it < nsb:
                    conv2a_stage(it)

            # replica rows 64-127 = +1 x-column shift (= +256 elements);
            # duplicate the last position block so zero-padded single-tap
            # matmuls read finite data there.
            nc.vector.tensor_copy(h2brep[64:128, 0:169 * 256 - 256],
                                  h2brep[0:64, 256:169 * 256])
            nc.vector.tensor_copy(h2brep[64:128, 168 * 256:169 * 256],
                                  h2brep[0:64, 168 * 256:169 * 256])

            # ============ phase 2: locally-connected stack ============
            # ---- lconv3: 13x13 -> 9x9, 5x5 taps ----
            for p in range(81):
                i, j = p // 9, p % 9
                lwt = lwp.tile([128, 640], bf16, tag="lw3p")
                nc.sync.dma_start(lwt[:], lw3p_d[p])
                lws = lwp.tile([128, 320], bf16, tag="lw3s")
                nc.sync.dma_start(lws[:], lw3s_d[p])
                ps = lps.tile([64, 256], f32, tag="lps")
                for q in range(10):        # (u, v0/v0+1) pairs, K=128
                    u, v0 = q // 2, 2 * (q % 2)
                    col = ((i + u) * 13 + (j + v0)) * 256
                    nc.tensor.matmul(ps[:], lwt[:, 64 * q:64 * (q + 1)],
                                     h2brep[:, col:col + 256],
                                     start=(q == 0), stop=False)
                for s in range(5):         # (u, v=4) singles, zero-padded K=128
                    col = ((i + s) * 13 + (j + 4)) * 256
                    nc.tensor.matmul(ps[:], lws[:, 64 * s:64 * (s + 1)],
                                     h2brep[:, col:col + 256],
                                     start=False, stop=(s == 4))
                nc.scalar.activation(h3rep[0:64, 256 * p:256 * (p + 1)], ps[:],
                                     Tanh, bias=lb3t[:, p:p + 1])
            nc.vector.tensor_copy(h3rep[64:128, 0:81 * 256 - 256],
                                  h3rep[0:64, 256:81 * 256])
            nc.vector.tensor_copy(h3rep[64:128, 80 * 256:81 * 256],
                                  h3rep[0:64, 80 * 256:81 * 256])

            # ---- lconv4: 9x9 -> 5x5, 5x5 taps ----
            for p in range(25):
                i, j = p // 5, p % 5
                lwt = lwp.tile([128, 640], bf16, tag="lw4p")
                nc.sync.dma_start(lwt[:], lw4p_d[p])
                lws = lwp.tile([128, 320], bf16, tag="lw4s")
                nc.sync.dma_start(lws[:], lw4s_d[p])
                ps = lps.tile([64, 256], f32, tag="lps")
                for q in range(10):
                    u, v0 = q // 2, 2 * (q % 2)
                    col = ((i + u) * 9 + (j + v0)) * 256
                    nc.tensor.matmul(ps[:], lwt[:, 64 * q:64 * (q + 1)],
                                     h3rep[:, col:col + 256],
                                     start=(q == 0), stop=False)
                for s in range(5):
                    col = ((i + s) * 9 + (j + 4)) * 256
                    nc.tensor.matmul(ps[:], lws[:, 64 * s:64 * (s + 1)],
                                     h3rep[:, col:col + 256],
                                     start=False, stop=(s == 4))
                nc.scalar.activation(h4rep[0:64, 256 * p:256 * (p + 1)], ps[:],
                                     Tanh, bias=lb4t[:, p:p + 1])
            nc.vector.tensor_copy(h4rep[64:128, 0:25 * 256 - 256],
                                  h4rep[0:64, 256:25 * 256])
            nc.vector.tensor_copy(h4rep[64:128, 24 * 256:25 * 256],
                                  h4rep[0:64, 24 * 256:25 * 256])

            # ---- lconv5: 5x5 -> 3x3, 3x3 taps ----
            for p in range(9):
                i, j = p // 3, p % 3
                lwt = lwp.tile([128, 192], bf16, tag="lw5p")
                nc.sync.dma_start(lwt[:], lw5p_d[p])
                lws = lwp.tile([128, 192], bf16, tag="lw5s")
                nc.sync.dma_start(lws[:], lw5s_d[p])
                ps = lps.tile([64, 256], f32, tag="lps")
                for q in range(3):         # (u, v=0/1) pairs
                    col = ((i + q) * 5 + (j + 0)) * 256
                    nc.tensor.matmul(ps[:], lwt[:, 64 * q:64 * (q + 1)],
                                     h4rep[:, col:col + 256],
                                     start=(q == 0), stop=False)
                for s in range(3):         # (u, v=2) singles, zero-padded K=128
                    col = ((i + s) * 5 + (j + 2)) * 256
                    nc.tensor.matmul(ps[:], lws[:, 64 * s:64 * (s + 1)],
                                     h4rep[:, col:col + 256],
                                     start=False, stop=(s == 2))
                nc.scalar.activation(h5t[:, 256 * p:256 * (p + 1)], ps[:],
                                     Tanh, bias=lb5t[:, p:p + 1])

            # ---- head: logits[o, b] = sum_f hw[o, f] feat[f, b] (h5 part) ----
            psh = hps.tile([2, 256], f32)
            for yx in range(9):
                nc.tensor.matmul(psh[:], hwch[:, 2 * yx:2 * yx + 2],
                                 h5t[:, 256 * yx:256 * (yx + 1)],
                                 start=(yx == 0), stop=(yx == 8))
            lg = wp.tile([2, 256], f32)
            nc.vector.tensor_copy(lg[:], psh[:])
            nc.sync.dma_start(logits_d[:], lg[:])

    nc.compile()
    return nc


def _prep_weights(w1, b1, w2a, b2a, w2b, b2b, lw3, lb3, lw4, lb4, lw5, lb5, hw):
    """Host-side reshape of weights into the on-chip matmul layouts."""
    out = {}
    # conv1 quadrant weights: 4 MMs, rows = plane*20 + g*5 + ci, cols g*32+co
    # MM0: taps (0,0)@P0 (0,1)@P1 (1,1)@P2 (1,0)@P3  at (y, x)
    # MM1: taps (2,0)@P0 (2,1)@P1                    at (y+1, x)
    # MM2: taps (1,2)@P3 (0,2)@P4                    at (y, x+1)
    # MM3: tap  (2,2)@P4                             at (y+1, x+1)
    w1q = np.zeros((4, 128, 128), np.float32)
    mm_taps = [
        [(0, (0, 0)), (1, (0, 1)), (2, (1, 1)), (3, (1, 0))],
        [(0, (2, 0)), (1, (2, 1))],
        [(3, (1, 2)), (4, (0, 2))],
        [(4, (2, 2))],
    ]
    for m, taps in enumerate(mm_taps):
        for pl, (di, dj) in taps:
            for g in range(4):
                w1q[m, 20 * pl + 5 * g:20 * pl + 5 * g + 5,
                    32 * g:32 * (g + 1)] = w1[:, :, di, dj].T
    out["w1q"] = np.ascontiguousarray(
        w1q.transpose(1, 0, 2).reshape(128, 4 * 128)).astype(BF)
    out["b1t"] = np.tile(b1, 4)[:, None].astype(np.float32)

    # conv2a tap-pair weights: rows rep*64 + gl*32 + ci, cols gl*64 + co
    w2ap = np.zeros((3, 128, 128), np.float32)
    w2as = np.zeros((3, 128, 128), np.float32)
    for di in range(3):
        for gl in range(2):
            for rep in range(2):
                w2ap[di, 64 * rep + 32 * gl:64 * rep + 32 * (gl + 1),
                     64 * gl:64 * (gl + 1)] = w2a[:, :, di, rep].T
            w2as[di, 32 * gl:32 * (gl + 1),
                 64 * gl:64 * (gl + 1)] = w2a[:, :, di, 2].T
    out["w2ap"] = np.ascontiguousarray(
        w2ap.transpose(1, 0, 2).reshape(128, 3 * 128)).astype(BF)
    out["w2as"] = np.ascontiguousarray(
        w2as.transpose(1, 0, 2).reshape(128, 3 * 128)).astype(BF)
    out["b2at"] = np.tile(b2a, 2)[:, None].astype(np.float32)

    w2bbd = np.zeros((128, 9, 128), np.float32)
    for t, (di, dj) in enumerate(TAPS3):
        blk = w2b[:, :, di, dj].T         # [64ci, 64co]
        for g2 in range(2):
            w2bbd[64 * g2:64 * (g2 + 1), t, 64 * g2:64 * (g2 + 1)] = blk
    out["w2bbd"] = w2bbd.reshape(128, 9 * 128).astype(BF)
    out["b2bt"] = np.tile(b2b, 2)[:, None].astype(np.float32)

    def lc_pack(lw, Ho, Wo, kh, kw):
        npos = Ho * Wo
        npair = kh * (kw // 2)
        nsing = kh
        lp = np.zeros((npos, 128, npair * 64), np.float32)
        ls = np.zeros((npos, 128, nsing * 64), np.float32)
        for p in range(npos):
            i, j = p // Wo, p % Wo
            for q in range(npair):
                u, v0 = q // (kw // 2), 2 * (q % (kw // 2))
                lp[p, 0:64, 64 * q:64 * (q + 1)] = lw[i, j, :, :, u, v0].T
                lp[p, 64:128, 64 * q:64 * (q + 1)] = lw[i, j, :, :, u, v0 + 1].T
            for s in range(nsing):
                ls[p, 0:64, 64 * s:64 * (s + 1)] = lw[i, j, :, :, s, kw - 1].T
        return lp.astype(BF), ls.astype(BF)

    out["lw3p"], out["lw3s"] = lc_pack(lw3, 9, 9, 5, 5)
    out["lb3t"] = np.ascontiguousarray(
        lb3.transpose(2, 0, 1).reshape(64, 81)).astype(np.float32)
    out["lw4p"], out["lw4s"] = lc_pack(lw4, 5, 5, 5, 5)
    out["lb4t"] = np.ascontiguousarray(
        lb4.transpose(2, 0, 1).reshape(64, 25)).astype(np.float32)
    out["lw5p"], out["lw5s"] = lc_pack(lw5, 3, 3, 3, 3)
    out["lb5t"] = np.ascontiguousarray(
        lb5.transpose(2, 0, 1).reshape(64, 9)).astype(np.float32)

    # head: feature f = co*9 + yx; chunk yx -> [64co, 2]
    out["hwch"] = np.ascontiguousarray(
        hw[:, :576].reshape(2, 64, 9).transpose(1, 2, 0).reshape(64, 18)
    ).astype(BF)
    return out


def _prep_x(x):
    """Quadrant-split + per-sub-batch arrangement of x for all cores.

    Returns [N_CORES, NSB, 128, 1800] bf16: partition rows = plane*20 +
    g*5 + ci (planes h0w0, h0w1, h1w1, h1w0, h0w0-dup; rows 100-127
    zero), cols = b*900 + yi*30 + xi.
    """
    planes = np.stack([x[:, :, hp::2, wp::2] for (hp, wp) in Q_PLANES],
                      axis=1)                     # [B, 5pl, 5ci, 30, 30]
    planes = planes.reshape(B_FULL, 5, 5, 900)
    # batch index = core*256 + g*64 + sb*BSB + b
    a = planes.reshape(N_CORES, 4, NSB, BSB, 5, 5, 900)
    a = a.transpose(0, 2, 4, 1, 5, 3, 6)          # core, sb, pl, g, ci, b, m
    a = a.reshape(N_CORES, NSB, 100, BSB * 900)
    out = np.zeros((N_CORES, NSB, 128, BSB * 900), np.float32)
    out[:, :, :100, :] = a
    return out.astype(BF)


def kernel(x, info, w1, b1, w2a, b2a, w2b, b2b, lw3, lb3, lw4, lb4, lw5, lb5,
           hw, hb, _trace=False):
    x = np.asarray(x, np.float32)
    if "nc" not in _CACHE:
        _CACHE["nc"] = _build_module()
    nc = _CACHE["nc"]

    wts = _prep_weights(np.asarray(w1, np.float32), np.asarray(b1, np.float32),
                        np.asarray(w2a, np.float32), np.asarray(b2a, np.float32),
                        np.asarray(w2b, np.float32), np.asarray(b2b, np.float32),
                        np.asarray(lw3, np.float32), np.asarray(lb3, np.float32),
                        np.asarray(lw4, np.float32), np.asarray(lb4, np.float32),
                        np.asarray(lw5, np.float32), np.asarray(lb5, np.float32),
                        np.asarray(hw, np.float32))

    xq = _prep_x(x)
    in_maps = []
    for c in range(N_CORES):
        m = dict(wts)
        m["xq"] = xq[c]
        in_maps.append(m)

    res = bass_utils.run_bass_kernel_spmd(
        nc, in_maps, core_ids=list(range(N_CORES)), trace=_trace)
    if _trace:
        _CACHE["last_results"] = res

    logits = np.concatenate(
        [res.results[c]["logits"].T for c in range(N_CORES)], axis=0)  # [2048, 2]

    # host-side tail: info contribution + bias + softmax (fp32)
    info = np.asarray(info, np.float32)
    hw = np.asarray(hw, np.float32)
    hb = np.asarray(hb, np.float32)
    logits = logits + info @ hw[:, 576:].T + hb[None, :]
    m = logits.max(axis=1, keepdims=True)
    e = np.exp(logits - m)
    return (e / e.sum(axis=1, keepdims=True)).astype(np.float32)
